# revision 17
# baseline (speedup 1.0000x reference)
"""Trainium2 Bass kernel for nn_BasicRGCN (2-layer RGCN + DistMult scoring).

Distribution strategy (8 NeuronCores, one chip):
  - Graph-row sharding: core k owns rows [512k, 512k+512) of the node set.
    Each core computes its row-chunk of both RGCN layers over ALL relations,
    accumulating the relation sum exactly in fp32 PSUM (no AllReduce needed).
  - Between layers, the per-core H1 chunks (fp16) are AllGathered so every
    core has the full H1 for layer 2.
  - c is folded into A on the host (c_r * (A_r H W_r^T) == (c_r*A_r) H W_r^T).
  - A@H precision: A is a single fp16 operand; H is fp16 with an optional
    second lo-pass (PASSES_L* = 2 -> ~fp32-exact, 1 -> fastest). The H-side
    rounding dominates the end-to-end error; A's fp16 rounding is negligible
    (measured H2 rel err 6.5e-6 at 2-pass, 1.8e-4 at 1-pass).
  - W-projection precision (W_MODE): "f16x3" runs aht@W^T as a scaled fp16
    hi/lo 3-pass (hi*Whi + lo*Whi + hi*Wlo, ~2^-21 operand error, fp16
    throughput); "f32" is exact but 4 cyc/row with a serialized 4-byte
    weight load; "f32r" is TF32-like 1 cyc/row (adds ~1.7e-4 H2 error).
    Layer-2 aht magnitudes (~2e6) overflow fp16, so the f16x3 path folds a
    2^-8 scale into W1 on the host (H1 and H2 come out scaled on device;
    the host rescales H2 after download — power-of-two scaling is exact).
  - A is pre-tiled on the host to [128, R, KT, CH] (partition-major), so each
    relation's chunk loads with one 4 MiB DMA of fully contiguous
    32 KiB-per-partition descriptors, instead of a 1 KiB-descriptor spray.
  - DistMult scoring (0.01% of the FLOPs, gather-bound) runs on the host
    from the device-computed H2 in float64, then sigmoid.

Measured on the target (per 2-layer forward, 8 cores):
  PASSES=(2,2), W_MODE=f16x3: score rel err 0.0 vs float64 reference.
"""

import numpy as np

R, N, F, B = 8, 4096, 256, 16384
N_CORES = 8
CH = N // N_CORES          # 512 rows per core
KT = N // 128              # 32 contraction k-tiles
NT = CH // 128             # 4 output row-tiles per chunk
PASSES_L1 = 2              # H0 passes (2 = hi+lo fp16, 1 = hi only)
PASSES_L2 = 2              # H1 passes
W_MODE = "f32"             # "f16x3" | "f32" | "f32r"
L2_SCALE = 2.0 ** -8       # pre-split scale for layer-2 aht (f16x3 only)

_programs = {}


def _build(reps=1, p1=PASSES_L1, p2=PASSES_L2, wmode=W_MODE):
    import concourse.bacc as bacc
    import concourse.tile as tile
    import concourse.mybir as mybir

    f16 = mybir.dt.float16
    f32 = mybir.dt.float32
    f16x3 = wmode == "f16x3"
    wdt = {"f16x3": f16, "f32": f32, "f32r": mybir.dt.float32r}[wmode]

    nc = bacc.Bacc("TRN2", target_bir_lowering=False, debug=False,
                   num_devices=N_CORES)

    at_d = nc.dram_tensor("at", [128, R, KT, CH], f16, kind="ExternalInput")
    h0_d = nc.dram_tensor("h0", [128, KT, p1, F], f16, kind="ExternalInput")
    if f16x3:
        # host-packed tile layout [128, hi/lo, R, ftin, F]
        w1t_d = nc.dram_tensor("w1t", [128, 2, R, 2, F], f16, kind="ExternalInput")
        w2t_d = nc.dram_tensor("w2t", [128, 2, R, 2, F], f16, kind="ExternalInput")
    else:
        w1t_d = nc.dram_tensor("w1t", [R, F, F], wdt, kind="ExternalInput")
        w2t_d = nc.dram_tensor("w2t", [R, F, F], wdt, kind="ExternalInput")
    h2_d = nc.dram_tensor("h2", [CH, F], f32, kind="ExternalOutput")

    groups = [list(range(N_CORES))]
    two_pass = max(p1, p2) == 2

    with tile.TileContext(nc) as tc:
        with (
            tc.tile_pool(name="hpool", bufs=1 if two_pass else 2) as hpool,
            tc.tile_pool(name="apool",
                         bufs=2 if (two_pass and f16x3) else 3) as apool,
            tc.tile_pool(name="wpool", bufs=1) as wpool,
            tc.tile_pool(name="ahtp", bufs=2) as ahtp,
            tc.tile_pool(name="hout", bufs=1) as hout,
            tc.tile_pool(name="ps_aht", bufs=4, space="PSUM") as ps_aht,
            tc.tile_pool(name="ps_y", bufs=1, space="PSUM") as ps_y,
            tc.tile_pool(name="dram", bufs=1, space="DRAM") as dram,
        ):
            # persistent W tiles (tiny, loaded once)
            if f16x3:
                w1 = wpool.tile([128, 2, R, 2, F], f16, tag="w1")
                w2 = wpool.tile([128, 2, R, 2, F], f16, tag="w2")
                nc.gpsimd.dma_start(w1[:], w1t_d[:])
                nc.gpsimd.dma_start(w2[:], w2t_d[:])
            else:
                w1 = wpool.tile([128, R, 2, F], wdt, tag="w1")
                w2 = wpool.tile([128, R, 2, F], wdt, tag="w2")
                nc.gpsimd.dma_start(
                    w1[:], w1t_d.rearrange("r (ft p) o -> p r ft o", p=128)[:])
                nc.gpsimd.dma_start(
                    w2[:], w2t_d.rearrange("r (ft p) o -> p r ft o", p=128)[:])

            def emit_layer(h_t, w_t, passes, li):
                """h_t: [128, KT, >=passes, F] fp16. Returns y PSUM tiles."""
                y_ps = [ps_y.tile([128, F], f32, tag=f"y{nt}", name=f"y{li}_{nt}")
                        for nt in range(NT)]

                def emit_y(r, evac):
                    if f16x3:
                        s1, slo = evac
                        for nt in range(NT):
                            ns = slice(nt * 128, nt * 128 + 128)
                            for ft in range(2):
                                first = r == 0 and ft == 0
                                nc.tensor.matmul(
                                    y_ps[nt][:], s1[:, ft, ns], w_t[:, 0, r, ft, :],
                                    start=first, stop=False)
                                nc.tensor.matmul(
                                    y_ps[nt][:], slo[:, ft, ns], w_t[:, 0, r, ft, :],
                                    start=False, stop=False)
                                nc.tensor.matmul(
                                    y_ps[nt][:], s1[:, ft, ns], w_t[:, 1, r, ft, :],
                                    start=False,
                                    stop=(r == R - 1 and ft == 1))
                    else:
                        (aht_s,) = evac
                        for nt in range(NT):
                            ns = slice(nt * 128, nt * 128 + 128)
                            for ft in range(2):
                                nc.tensor.matmul(
                                    y_ps[nt][:], aht_s[:, ft, ns], w_t[:, r, ft, :],
                                    start=(r == 0 and ft == 0),
                                    stop=(r == R - 1 and ft == 1))

                pending = None
                for r in range(R):
                    a = apool.tile([128, KT, CH], f16, tag="a", name=f"a{li}_{r}")
                    nc.sync.dma_start(a[:], at_d[:, r, :, :])

                    aht_ps = [ps_aht.tile([128, CH], f32, tag="aht",
                                          name=f"aht{li}_{r}_{ft2}") for ft2 in range(2)]
                    for ft in range(2):
                        fs = slice(ft * 128, ft * 128 + 128)
                        for kt in range(KT):
                            nc.tensor.matmul(aht_ps[ft][:], h_t[:, kt, 0, fs],
                                             a[:, kt, :], start=(kt == 0),
                                             stop=(passes == 1 and kt == KT - 1))
                            if passes == 2:
                                nc.tensor.matmul(aht_ps[ft][:], h_t[:, kt, 1, fs],
                                                 a[:, kt, :], start=False,
                                                 stop=(kt == KT - 1))
                    if f16x3:
                        # fp16 hi/lo split of aht (layer-2 range kept in fp16
                        # bounds by the host-side 2^-8 scale folded into W1)
                        t = ahtp.tile([128, 2, CH], f32, tag="aht_t")
                        for ft in range(2):
                            nc.vector.tensor_copy(t[:, ft, :], aht_ps[ft][:])
                        s1 = ahtp.tile([128, 2, CH], f16, tag="aht_s1")
                        nc.vector.tensor_copy(s1[:], t[:])
                        s32 = ahtp.tile([128, 2, CH], f32, tag="aht_s32")
                        nc.vector.tensor_copy(s32[:], s1[:])
                        slo = ahtp.tile([128, 2, CH], f16, tag="aht_slo")
                        nc.vector.tensor_sub(slo[:], t[:], s32[:])
                        evac = (s1, slo)
                    else:
                        aht_s = ahtp.tile([128, 2, CH], wdt, tag="aht_s")
                        for ft in range(2):
                            nc.vector.tensor_copy(aht_s[:, ft, :], aht_ps[ft][:])
                        evac = (aht_s,)
                    if pending is not None:
                        emit_y(*pending)
                    pending = (r, evac)
                emit_y(*pending)
                return y_ps

            for rep in range(reps):
                # ---- layer 1: H0 tiles from the host-packed input ----
                ht = hpool.tile([128, KT, max(p1, p2), F], f16, tag="ht",
                                name=f"ht1_{rep}")
                nc.sync.dma_start(ht[:, :, 0:p1, :], h0_d[:])

                y_ps = emit_layer(ht, w1, p1, li=f"{rep}a")

                # cast H1 chunk to fp16 (hi, and lo if 2-pass) and AllGather.
                # bb rank layout [NT, 128, p2, F]; AllGather concat on axis 0
                # -> gag [KT, 128, p2, F] == full H1 in tile layout
                h1f = hout.tile([128, NT, F], f32, tag="h1f")
                for nt in range(NT):
                    nc.vector.tensor_copy(h1f[:, nt, :], y_ps[nt][:])
                h1h = hout.tile([128, NT, F], f16, tag="h1h")
                nc.vector.tensor_copy(h1h[:], h1f[:])
                bb = dram.tile([NT, 128, p2, F], f16, tag="bb")
                bv = bb.rearrange("q p j f -> p q j f")
                nc.gpsimd.dma_start(bv[:, :, 0, :], h1h[:])
                if p2 == 2:
                    h1h32 = hout.tile([128, NT, F], f32, tag="h1h32")
                    nc.vector.tensor_copy(h1h32[:], h1h[:])
                    h1l = hout.tile([128, NT, F], f16, tag="h1l")
                    nc.vector.tensor_sub(h1l[:], h1f[:], h1h32[:])
                    nc.gpsimd.dma_start(bv[:, :, 1, :], h1l[:])
                gag = dram.tile([KT, 128, p2, F], f16, tag="gag", addr_space="Shared")
                nc.gpsimd.collective_compute(
                    "AllGather", mybir.AluOpType.bypass,
                    replica_groups=groups, ins=[bb.opt()], outs=[gag.opt()])

                ht2 = hpool.tile([128, KT, max(p1, p2), F], f16, tag="ht",
                                 name=f"ht2_{rep}")
                nc.sync.dma_start(ht2[:, :, 0:p2, :],
                                  gag.rearrange("kt p j f -> p kt j f")[:])

                # ---- layer 2 ----
                y_ps2 = emit_layer(ht2, w2, p2, li=f"{rep}b")
                h2f = hout.tile([128, NT, F], f32, tag="h2f")
                for nt in range(NT):
                    nc.vector.tensor_copy(h2f[:, nt, :], y_ps2[nt][:])
                nc.gpsimd.dma_start(
                    h2_d.rearrange("(nt p) f -> p nt f", p=128)[:], h2f[:])

    nc.compile()
    return nc


def _get_program(reps=1, p1=PASSES_L1, p2=PASSES_L2, wmode=W_MODE):
    key = (reps, p1, p2, wmode)
    if key not in _programs:
        _programs[key] = _build(reps, p1, p2, wmode)
    return _programs[key]


def _prepare_in_maps(adjacency, features, c, W1, W2, p1=PASSES_L1,
                     wmode=W_MODE):
    h0 = np.ascontiguousarray(features, dtype=np.float32)
    h0_hi = h0.astype(np.float16)
    h0p = np.empty((128, KT, p1, F), dtype=np.float16)
    h0p[:, :, 0, :] = h0_hi.reshape(KT, 128, F).transpose(1, 0, 2)
    if p1 == 2:
        h0_lo = (h0 - h0_hi.astype(np.float32)).astype(np.float16)
        h0p[:, :, 1, :] = h0_lo.reshape(KT, 128, F).transpose(1, 0, 2)

    def pack_w(W, scale=1.0):
        wt = np.ascontiguousarray(W.transpose(0, 2, 1), dtype=np.float32)
        if wmode != "f16x3":
            return wt
        wt = wt * np.float32(scale)
        hi = wt.astype(np.float16)
        lo = (wt - hi.astype(np.float32)).astype(np.float16)
        # tile layout [128, hi/lo, R, ftin, F]: w[p,j,r,ft,o] = W^T[r, ft*128+p, o]
        out = np.empty((128, 2, R, 2, F), dtype=np.float16)
        for j, h in enumerate((hi, lo)):
            out[:, j] = h.reshape(R, 2, 128, F).transpose(2, 0, 1, 3)
        return out

    # fold the layer-2 fp16 range scale into W1: the device computes
    # H1_dev = L2_SCALE * H1 and y2_dev = L2_SCALE * y2 (power of two, exact);
    # the host multiplies H2 back by 1/L2_SCALE after download.
    w1t = pack_w(W1, L2_SCALE if wmode == "f16x3" else 1.0)
    w2t = pack_w(W2)

    # A tiles: At[p, r, kt, j] = (c*A)[r, chunk_start + j, kt*128 + p] as fp16
    a16 = np.empty((128, R, KT, N), dtype=np.float16)
    for r in range(R):
        X = (adjacency[r] * c[r]).T.astype(np.float16)        # [N(contract), N(rows)]
        a16[:, r] = X.reshape(KT, 128, N).transpose(1, 0, 2)

    in_maps = []
    for k in range(N_CORES):
        at = np.ascontiguousarray(a16[:, :, :, k * CH:(k + 1) * CH])
        in_maps.append({
            "at": at, "h0": h0p, "w1t": w1t, "w2t": w2t,
        })
    return in_maps


def _run_device(in_maps, reps=1, p1=PASSES_L1, p2=PASSES_L2, wmode=W_MODE):
    from concourse.bass_utils import run_bass_kernel_spmd
    nc = _get_program(reps, p1, p2, wmode)
    res = run_bass_kernel_spmd(nc, in_maps, core_ids=list(range(N_CORES)))
    H2 = np.concatenate([res.results[k]["h2"] for k in range(N_CORES)], axis=0)
    if wmode == "f16x3":
        H2 = H2 * np.float32(1.0 / L2_SCALE)
    return H2


def _score_host(H2, rel_mats, e1_idx, rel_idx, e2_idx):
    E1 = H2[e1_idx].astype(np.float64)
    E2 = H2[e2_idx].astype(np.float64)
    Mm = np.asarray(rel_mats, dtype=np.float64)
    idx = np.arange(F)
    offdiag = Mm.copy()
    offdiag[:, idx, idx] = 0.0
    if not offdiag.any():
        mdiag = Mm[:, idx, idx]
        scores = np.einsum("bf,bf,bf->b", E1, mdiag[rel_idx], E2)
    else:
        scores = np.empty(E1.shape[0], dtype=np.float64)
        for r in range(R):
            m = rel_idx == r
            if m.any():
                scores[m] = np.einsum("bf,fg,bg->b", E1[m], Mm[r], E2[m])
    out = np.empty_like(scores)
    pos = scores >= 0
    out[pos] = 1.0 / (1.0 + np.exp(-scores[pos]))
    ez = np.exp(scores[~pos])
    out[~pos] = ez / (1.0 + ez)
    return out.astype(np.float32)


def kernel(adjacency, features, c, W1, W2, rel_mats, e1_idx, rel_idx, e2_idx,
           _reps=1):
    adjacency = np.asarray(adjacency, dtype=np.float32)
    features = np.asarray(features, dtype=np.float32)
    c = np.asarray(c, dtype=np.float32)
    W1 = np.asarray(W1, dtype=np.float32)
    W2 = np.asarray(W2, dtype=np.float32)
    rel_mats = np.asarray(rel_mats, dtype=np.float32)
    e1_idx = np.asarray(e1_idx)
    rel_idx = np.asarray(rel_idx)
    e2_idx = np.asarray(e2_idx)

    in_maps = _prepare_in_maps(adjacency, features, c, W1, W2)
    H2 = _run_device(in_maps, reps=_reps)
    return _score_host(H2, rel_mats, e1_idx, rel_idx, e2_idx)


# revision 22
# speedup vs baseline: 1.5737x; 1.5737x over previous
"""Trainium2 Bass kernel for nn_BasicRGCN (2-layer RGCN + DistMult scoring).

Distribution strategy (8 NeuronCores, one chip):
  - Graph-row sharding: core k owns rows [512k, 512k+512) of the node set.
    Each core computes its row-chunk of both RGCN layers over ALL relations,
    accumulating the relation sum exactly in fp32 PSUM (no AllReduce needed).
  - Between layers, the per-core H1 chunks (fp16) are AllGathered so every
    core has the full H1 for layer 2.
  - c is folded into A on the host (c_r * (A_r H W_r^T) == (c_r*A_r) H W_r^T).
  - A@H precision: A is a single fp16 operand; H is fp16 with an optional
    second lo-pass (PASSES_L* = 2 -> ~fp32-exact, 1 -> fastest). The H-side
    rounding dominates the end-to-end error; A's fp16 rounding is negligible
    (measured H2 rel err 6.5e-6 at 2-pass, 1.8e-4 at 1-pass).
  - W-projection precision (W_MODE): "f16x3" runs aht@W^T as a scaled fp16
    hi/lo 3-pass (hi*Whi + lo*Whi + hi*Wlo, ~2^-21 operand error, fp16
    throughput); "f32" is exact but 4 cyc/row with a serialized 4-byte
    weight load; "f32r" is TF32-like 1 cyc/row (adds ~1.7e-4 H2 error).
    Layer-2 aht magnitudes (~2e6) overflow fp16, so the f16x3 path folds a
    2^-8 scale into W1 on the host (H1 and H2 come out scaled on device;
    the host rescales H2 after download — power-of-two scaling is exact).
  - A is pre-tiled on the host to [128, R, KT, CH] (partition-major), so each
    relation's chunk loads with one 4 MiB DMA of fully contiguous
    32 KiB-per-partition descriptors, instead of a 1 KiB-descriptor spray.
  - DistMult scoring (0.01% of the FLOPs, gather-bound) runs on the host
    from the device-computed H2 in float64, then sigmoid.

Measured on the target (per 2-layer forward, 8 cores):
  PASSES=(2,2), W_MODE=f16x3: score rel err 0.0 vs float64 reference.
"""

import numpy as np

R, N, F, B = 8, 4096, 256, 16384
N_CORES = 8
CH = N // N_CORES          # 512 rows per core
KT = N // 128              # 32 contraction k-tiles
NT = CH // 128             # 4 output row-tiles per chunk
PASSES_L1 = 2              # H0 passes (2 = hi+lo fp16, 1 = hi only)
PASSES_L2 = 2              # H1 passes
W_MODE = "f16x3"           # "f16x3" | "f32" | "f32r"
L2_SCALE = 2.0 ** -8       # pre-split scale for layer-2 aht (f16x3 only)

_programs = {}


def _build(reps=1, p1=PASSES_L1, p2=PASSES_L2, wmode=W_MODE, bufcfg=None):
    import concourse.bacc as bacc
    import concourse.tile as tile
    import concourse.mybir as mybir

    f16 = mybir.dt.float16
    f32 = mybir.dt.float32
    f16x3 = wmode == "f16x3"
    wdt = {"f16x3": f16, "f32": f32, "f32r": mybir.dt.float32r}[wmode]

    nc = bacc.Bacc("TRN2", target_bir_lowering=False, debug=False,
                   num_devices=N_CORES)

    at_d = nc.dram_tensor("at", [128, R, KT, CH], f16, kind="ExternalInput")
    h0_d = nc.dram_tensor("h0", [128, KT, p1, F], f16, kind="ExternalInput")
    if f16x3:
        # host-packed tile layout [128, hi/lo, R, ftin, F]
        w1t_d = nc.dram_tensor("w1t", [128, 2, R, 2, F], f16, kind="ExternalInput")
        w2t_d = nc.dram_tensor("w2t", [128, 2, R, 2, F], f16, kind="ExternalInput")
    else:
        w1t_d = nc.dram_tensor("w1t", [R, F, F], wdt, kind="ExternalInput")
        w2t_d = nc.dram_tensor("w2t", [R, F, F], wdt, kind="ExternalInput")
    h2_d = nc.dram_tensor("h2", [CH, F], f32, kind="ExternalOutput")

    groups = [list(range(N_CORES))]
    two_pass = max(p1, p2) == 2

    if two_pass and f16x3:
        a_bufs, h_bufs = 2, 2      # measured best: cross-rep ht prefetch
    elif two_pass:
        a_bufs, h_bufs = 3, 1      # f32 W tiles leave no room for hpool=2
    else:
        a_bufs, h_bufs = 3, 2
    if bufcfg is not None:
        a_bufs, h_bufs = bufcfg

    with tile.TileContext(nc) as tc:
        with (
            tc.tile_pool(name="hpool", bufs=h_bufs) as hpool,
            tc.tile_pool(name="apool", bufs=a_bufs) as apool,
            tc.tile_pool(name="wpool", bufs=1) as wpool,
            tc.tile_pool(name="ahtp", bufs=2) as ahtp,
            tc.tile_pool(name="hout", bufs=1) as hout,
            tc.tile_pool(name="ps_aht", bufs=4, space="PSUM") as ps_aht,
            tc.tile_pool(name="ps_y", bufs=1, space="PSUM") as ps_y,
            tc.tile_pool(name="dram", bufs=1, space="DRAM") as dram,
        ):
            # persistent W tiles (tiny, loaded once)
            if f16x3:
                w1 = wpool.tile([128, 2, R, 2, F], f16, tag="w1")
                w2 = wpool.tile([128, 2, R, 2, F], f16, tag="w2")
                nc.gpsimd.dma_start(w1[:], w1t_d[:])
                nc.gpsimd.dma_start(w2[:], w2t_d[:])
            else:
                w1 = wpool.tile([128, R, 2, F], wdt, tag="w1")
                w2 = wpool.tile([128, R, 2, F], wdt, tag="w2")
                nc.gpsimd.dma_start(
                    w1[:], w1t_d.rearrange("r (ft p) o -> p r ft o", p=128)[:])
                nc.gpsimd.dma_start(
                    w2[:], w2t_d.rearrange("r (ft p) o -> p r ft o", p=128)[:])

            def emit_layer(h_t, w_t, passes, li):
                """h_t: [128, KT, >=passes, F] fp16. Returns y PSUM tiles."""
                y_ps = [ps_y.tile([128, F], f32, tag=f"y{nt}", name=f"y{li}_{nt}")
                        for nt in range(NT)]

                def emit_y(r, evac):
                    if f16x3:
                        s1, slo = evac
                        for nt in range(NT):
                            ns = slice(nt * 128, nt * 128 + 128)
                            for ft in range(2):
                                first = r == 0 and ft == 0
                                nc.tensor.matmul(
                                    y_ps[nt][:], s1[:, ft, ns], w_t[:, 0, r, ft, :],
                                    start=first, stop=False)
                                nc.tensor.matmul(
                                    y_ps[nt][:], slo[:, ft, ns], w_t[:, 0, r, ft, :],
                                    start=False, stop=False)
                                nc.tensor.matmul(
                                    y_ps[nt][:], s1[:, ft, ns], w_t[:, 1, r, ft, :],
                                    start=False,
                                    stop=(r == R - 1 and ft == 1))
                    else:
                        (aht_s,) = evac
                        for nt in range(NT):
                            ns = slice(nt * 128, nt * 128 + 128)
                            for ft in range(2):
                                nc.tensor.matmul(
                                    y_ps[nt][:], aht_s[:, ft, ns], w_t[:, r, ft, :],
                                    start=(r == 0 and ft == 0),
                                    stop=(r == R - 1 and ft == 1))

                pending = None
                for r in range(R):
                    a = apool.tile([128, KT, CH], f16, tag="a", name=f"a{li}_{r}")
                    nc.sync.dma_start(a[:], at_d[:, r, :, :])

                    aht_ps = [ps_aht.tile([128, CH], f32, tag="aht",
                                          name=f"aht{li}_{r}_{ft2}") for ft2 in range(2)]
                    for ft in range(2):
                        fs = slice(ft * 128, ft * 128 + 128)
                        for kt in range(KT):
                            nc.tensor.matmul(aht_ps[ft][:], h_t[:, kt, 0, fs],
                                             a[:, kt, :], start=(kt == 0),
                                             stop=(passes == 1 and kt == KT - 1))
                            if passes == 2:
                                nc.tensor.matmul(aht_ps[ft][:], h_t[:, kt, 1, fs],
                                                 a[:, kt, :], start=False,
                                                 stop=(kt == KT - 1))
                    if f16x3:
                        # fp16 hi/lo split of aht (layer-2 range kept in fp16
                        # bounds by the host-side 2^-8 scale folded into W1)
                        t = ahtp.tile([128, 2, CH], f32, tag="aht_t")
                        for ft in range(2):
                            nc.vector.tensor_copy(t[:, ft, :], aht_ps[ft][:])
                        s1 = ahtp.tile([128, 2, CH], f16, tag="aht_s1")
                        nc.vector.tensor_copy(s1[:], t[:])
                        s32 = ahtp.tile([128, 2, CH], f32, tag="aht_s32")
                        nc.vector.tensor_copy(s32[:], s1[:])
                        slo = ahtp.tile([128, 2, CH], f16, tag="aht_slo")
                        nc.vector.tensor_sub(slo[:], t[:], s32[:])
                        evac = (s1, slo)
                    else:
                        aht_s = ahtp.tile([128, 2, CH], wdt, tag="aht_s")
                        for ft in range(2):
                            nc.vector.tensor_copy(aht_s[:, ft, :], aht_ps[ft][:])
                        evac = (aht_s,)
                    if pending is not None:
                        emit_y(*pending)
                    pending = (r, evac)
                emit_y(*pending)
                return y_ps

            for rep in range(reps):
                # ---- layer 1: H0 tiles from the host-packed input ----
                ht = hpool.tile([128, KT, max(p1, p2), F], f16, tag="ht",
                                name=f"ht1_{rep}")
                nc.sync.dma_start(ht[:, :, 0:p1, :], h0_d[:])

                y_ps = emit_layer(ht, w1, p1, li=f"{rep}a")

                # cast H1 chunk to fp16 (hi, and lo if 2-pass) and AllGather.
                # bb rank layout [NT, 128, p2, F]; AllGather concat on axis 0
                # -> gag [KT, 128, p2, F] == full H1 in tile layout
                h1f = hout.tile([128, NT, F], f32, tag="h1f")
                for nt in range(NT):
                    nc.vector.tensor_copy(h1f[:, nt, :], y_ps[nt][:])
                h1h = hout.tile([128, NT, F], f16, tag="h1h")
                nc.vector.tensor_copy(h1h[:], h1f[:])
                bb = dram.tile([NT, 128, p2, F], f16, tag="bb")
                bv = bb.rearrange("q p j f -> p q j f")
                nc.gpsimd.dma_start(bv[:, :, 0, :], h1h[:])
                if p2 == 2:
                    h1h32 = hout.tile([128, NT, F], f32, tag="h1h32")
                    nc.vector.tensor_copy(h1h32[:], h1h[:])
                    h1l = hout.tile([128, NT, F], f16, tag="h1l")
                    nc.vector.tensor_sub(h1l[:], h1f[:], h1h32[:])
                    nc.gpsimd.dma_start(bv[:, :, 1, :], h1l[:])
                gag = dram.tile([KT, 128, p2, F], f16, tag="gag", addr_space="Shared")
                nc.gpsimd.collective_compute(
                    "AllGather", mybir.AluOpType.bypass,
                    replica_groups=groups, ins=[bb.opt()], outs=[gag.opt()])

                ht2 = hpool.tile([128, KT, max(p1, p2), F], f16, tag="ht",
                                 name=f"ht2_{rep}")
                nc.sync.dma_start(ht2[:, :, 0:p2, :],
                                  gag.rearrange("kt p j f -> p kt j f")[:])

                # ---- layer 2 ----
                y_ps2 = emit_layer(ht2, w2, p2, li=f"{rep}b")
                h2f = hout.tile([128, NT, F], f32, tag="h2f")
                for nt in range(NT):
                    nc.vector.tensor_copy(h2f[:, nt, :], y_ps2[nt][:])
                nc.gpsimd.dma_start(
                    h2_d.rearrange("(nt p) f -> p nt f", p=128)[:], h2f[:])

    nc.compile()
    return nc


def _get_program(reps=1, p1=PASSES_L1, p2=PASSES_L2, wmode=W_MODE, bufcfg=None):
    key = (reps, p1, p2, wmode, bufcfg)
    if key not in _programs:
        _programs[key] = _build(reps, p1, p2, wmode, bufcfg)
    return _programs[key]


def _prepare_in_maps(adjacency, features, c, W1, W2, p1=PASSES_L1,
                     wmode=W_MODE):
    h0 = np.ascontiguousarray(features, dtype=np.float32)
    h0_hi = h0.astype(np.float16)
    h0p = np.empty((128, KT, p1, F), dtype=np.float16)
    h0p[:, :, 0, :] = h0_hi.reshape(KT, 128, F).transpose(1, 0, 2)
    if p1 == 2:
        h0_lo = (h0 - h0_hi.astype(np.float32)).astype(np.float16)
        h0p[:, :, 1, :] = h0_lo.reshape(KT, 128, F).transpose(1, 0, 2)

    def pack_w(W, scale=1.0):
        wt = np.ascontiguousarray(W.transpose(0, 2, 1), dtype=np.float32)
        if wmode != "f16x3":
            return wt
        wt = wt * np.float32(scale)
        hi = wt.astype(np.float16)
        lo = (wt - hi.astype(np.float32)).astype(np.float16)
        # tile layout [128, hi/lo, R, ftin, F]: w[p,j,r,ft,o] = W^T[r, ft*128+p, o]
        out = np.empty((128, 2, R, 2, F), dtype=np.float16)
        for j, h in enumerate((hi, lo)):
            out[:, j] = h.reshape(R, 2, 128, F).transpose(2, 0, 1, 3)
        return out

    # fold the layer-2 fp16 range scale into W1: the device computes
    # H1_dev = L2_SCALE * H1 and y2_dev = L2_SCALE * y2 (power of two, exact);
    # the host multiplies H2 back by 1/L2_SCALE after download.
    w1t = pack_w(W1, L2_SCALE if wmode == "f16x3" else 1.0)
    w2t = pack_w(W2)

    # A tiles: At[p, r, kt, j] = (c*A)[r, chunk_start + j, kt*128 + p] as fp16
    a16 = np.empty((128, R, KT, N), dtype=np.float16)
    for r in range(R):
        X = (adjacency[r] * c[r]).T.astype(np.float16)        # [N(contract), N(rows)]
        a16[:, r] = X.reshape(KT, 128, N).transpose(1, 0, 2)

    in_maps = []
    for k in range(N_CORES):
        at = np.ascontiguousarray(a16[:, :, :, k * CH:(k + 1) * CH])
        in_maps.append({
            "at": at, "h0": h0p, "w1t": w1t, "w2t": w2t,
        })
    return in_maps


def _run_device(in_maps, reps=1, p1=PASSES_L1, p2=PASSES_L2, wmode=W_MODE):
    from concourse.bass_utils import run_bass_kernel_spmd
    nc = _get_program(reps, p1, p2, wmode)
    res = run_bass_kernel_spmd(nc, in_maps, core_ids=list(range(N_CORES)))
    H2 = np.concatenate([res.results[k]["h2"] for k in range(N_CORES)], axis=0)
    if wmode == "f16x3":
        H2 = H2 * np.float32(1.0 / L2_SCALE)
    return H2


def _score_host(H2, rel_mats, e1_idx, rel_idx, e2_idx):
    E1 = H2[e1_idx].astype(np.float64)
    E2 = H2[e2_idx].astype(np.float64)
    Mm = np.asarray(rel_mats, dtype=np.float64)
    idx = np.arange(F)
    offdiag = Mm.copy()
    offdiag[:, idx, idx] = 0.0
    if not offdiag.any():
        mdiag = Mm[:, idx, idx]
        scores = np.einsum("bf,bf,bf->b", E1, mdiag[rel_idx], E2)
    else:
        scores = np.empty(E1.shape[0], dtype=np.float64)
        for r in range(R):
            m = rel_idx == r
            if m.any():
                scores[m] = np.einsum("bf,fg,bg->b", E1[m], Mm[r], E2[m])
    out = np.empty_like(scores)
    pos = scores >= 0
    out[pos] = 1.0 / (1.0 + np.exp(-scores[pos]))
    ez = np.exp(scores[~pos])
    out[~pos] = ez / (1.0 + ez)
    return out.astype(np.float32)


def kernel(adjacency, features, c, W1, W2, rel_mats, e1_idx, rel_idx, e2_idx,
           _reps=1):
    adjacency = np.asarray(adjacency, dtype=np.float32)
    features = np.asarray(features, dtype=np.float32)
    c = np.asarray(c, dtype=np.float32)
    W1 = np.asarray(W1, dtype=np.float32)
    W2 = np.asarray(W2, dtype=np.float32)
    rel_mats = np.asarray(rel_mats, dtype=np.float32)
    e1_idx = np.asarray(e1_idx)
    rel_idx = np.asarray(rel_idx)
    e2_idx = np.asarray(e2_idx)

    in_maps = _prepare_in_maps(adjacency, features, c, W1, W2)
    H2 = _run_device(in_maps, reps=_reps)
    return _score_host(H2, rel_mats, e1_idx, rel_idx, e2_idx)
